# revision 34
# baseline (speedup 1.0000x reference)
"""MultiHeadAttn (post-LN, key-padding mask) Trainium2 Bass kernel, 8 cores.

Problem: h [S=2048, B=4, D=1024] f32; 16 heads x 64; key-padding mask [S, B];
out = LayerNorm(h + MHA(h)).

Sharding: core c handles batch b = c//2 and query half qh = c%2 (1024 query
rows), with all 16 heads and the full 2048-key context for that batch.
KV projections are recomputed by both cores of a batch pair (no collectives).

Per-core device pipeline (all matmuls bf16, fp32 accumulation in PSUM):
  - K^T/Q^T proj: stationary Wk/Wq column tiles, moving h^T -> [e, t] layout.
  - V proj: stationary h^T tiles, moving Wv -> natural [t, e] layout. The
    key-padding mask enters here only: V rows are scaled by z in {0,1}
    (masked key -> 0) and a z-column is appended per head (col 64), so the
    PV matmul (stationary [z*V | z], M=65) accumulates both the numerator
    and, in psum row 64, the softmax denominator sum_j z_j exp(s_qj).
  - Attention per head pair: scores^T [j,i] via row-paired matmuls (two heads
    in row strips 0-63 / 64-127 of the PE array), unmasked exp via ScalarE
    with 1/sqrt(dh) as the scale, then the M=65 PV above. No separate
    denominator matmuls (the baseline spent ~109us of PE on them).
  - Normalize: DVE reciprocal of psum row 64 -> GPSIMD partition_broadcast
    to 64 rows -> DVE psum*sbuf multiply into avt (head1 via a small
    partition-shift DMA into avt rows 64-127).
  - Output proj: stationary attn_vec^T tiles, moving Wo; residual add + LN
    fused on DVE/ScalarE.
Next head pair's K/Q projections are interleaved into the attention loop
(borrowing scores-pool PSUM slots) so the PE stays busy under the ACT-bound
softmax stream.
"""
import numpy as np
import ml_dtypes

N_HEAD, D_MODEL, D_HEAD = 16, 1024, 64
SEQ, BSZ = 2048, 4
QLEN = SEQ // 2
SCALE = 1.0 / D_HEAD ** 0.5
LN_EPS = 1e-5
P = 128
NSL = 512                   # matmul moving-operand slab (one PSUM bank fp32)
CT = D_MODEL // P           # 8 contraction tiles
ET = D_MODEL // P           # 8 e-tiles (2 heads each)
JT = SEQ // P               # 16 key tiles
JS = SEQ // NSL             # 4 key slabs
IS = QLEN // NSL            # 2 query slabs
TQ = QLEN // P              # 8 query-row tiles
HP = N_HEAD // 2            # 8 head pairs
DH1 = D_HEAD + 1            # V columns per head incl. the z (denominator) col

_CACHE = {}


def _build():
    from contextlib import ExitStack
    import concourse.bass as bass
    import concourse.mybir as mybir
    import concourse.tile as tile
    from concourse import bacc

    dt = mybir.dt
    f32, bf16 = dt.float32, dt.bfloat16
    AF = mybir.ActivationFunctionType
    ALU = mybir.AluOpType

    nc = bacc.Bacc(None, target_bir_lowering=False)

    hT = nc.dram_tensor("hT", [D_MODEL, SEQ], bf16, kind="ExternalInput")
    hq = nc.dram_tensor("hq", [QLEN, D_MODEL], f32, kind="ExternalInput")
    wq = nc.dram_tensor("wq", [D_MODEL, D_MODEL], bf16, kind="ExternalInput")
    wk = nc.dram_tensor("wk", [D_MODEL, D_MODEL], bf16, kind="ExternalInput")
    wv = nc.dram_tensor("wv", [D_MODEL, D_MODEL], bf16, kind="ExternalInput")
    wo = nc.dram_tensor("wo", [D_MODEL, D_MODEL], bf16, kind="ExternalInput")
    zt = nc.dram_tensor("zt", [SEQ], f32, kind="ExternalInput")
    gam = nc.dram_tensor("gam", [D_MODEL], f32, kind="ExternalInput")
    bet = nc.dram_tensor("bet", [D_MODEL], f32, kind="ExternalInput")
    out = nc.dram_tensor("out", [QLEN, D_MODEL], f32, kind="ExternalOutput")

    with tile.TileContext(nc) as tc, ExitStack() as ctx:
        persist = ctx.enter_context(tc.tile_pool(name="persist", bufs=1))

        # K/Q tiles die after their pair's scores — roll through 2 slots
        # instead of keeping all 8 pairs resident (saves 30KB of SBUF).
        ktq = ctx.enter_context(tc.tile_pool(name="ktq", bufs=2))
        v_sb = [persist.tile([P, N_HEAD, DH1], bf16, name=f"v{t}")
                for t in range(JT)]
        avt_sb = [persist.tile([P, QLEN], bf16, name=f"avt{e}") for e in range(ET)]
        z_sb = persist.tile([P, JT], f32, name="zmask")
        ones16 = persist.tile([P, N_HEAD, 1], f32, name="ones16")
        eps_sb = persist.tile([P, 1], f32, name="eps")

        nc.vector.memset(eps_sb, LN_EPS)
        nc.vector.memset(ones16, 1.0)

        nc.gpsimd.dma_start(out=z_sb,
                            in_=bass.AP(tensor=zt, offset=0, ap=[[1, P], [P, JT]]))

        # ---- phase-3 weights: load early into the region wvp freed ---------
        w3p = ctx.enter_context(tc.tile_pool(name="w3p", bufs=1))
        wo_sb = [w3p.tile([P, D_MODEL], bf16, name=f"wo{c}") for c in range(CT)]
        gam_sb = w3p.tile([P, D_MODEL], f32, name="gamr")
        bet_sb = w3p.tile([P, D_MODEL], f32, name="betr")

        # ---- phase 1 scope: h^T residency + streamed W columns --------------
        ph1_ctx = ExitStack()
        ph1 = ph1_ctx.enter_context(tc.tile_pool(name="ph1", bufs=1))
        ht_sb = [ph1.tile([P, SEQ], bf16, name=f"ht{c}") for c in range(CT)]

        wcol = ph1_ctx.enter_context(tc.tile_pool(name="wcol", bufs=3))

        def load_wcol(w, e, tag):
            wc = wcol.tile([P, CT, P], bf16, tag=tag, name=f"{tag}{e}")
            nc.sync.dma_start(
                out=wc,
                in_=w[:, e * P:(e + 1) * P].rearrange("(ct p) e -> p ct e", p=P))
            return wc

        # startup DMA priority: the first K-proj matmul needs wkc(0) + ht
        # tiles, so those go first on the sync queue; htq (needed later, for
        # Q-proj) goes via gpsimd SWDGE in parallel.
        wc0 = load_wcol(wk, 0, "wkc")
        ht_engs = [nc.sync, nc.scalar, nc.gpsimd]
        # column-half split, low halves first: the first K-proj slabs only
        # read columns 0-1023, so the PE can start ~half a warmup earlier.
        for half in range(2):
            cs = slice(half * SEQ // 2, (half + 1) * SEQ // 2)
            for c in range(CT):
                ht_engs[c % 3].dma_start(out=ht_sb[c][:, cs],
                                         in_=hT[c * P:(c + 1) * P, cs])

        def kq_group(ps_ap, wc, moving, sl):
            """8 accumulating matmuls: one K/Q-proj output group into psum."""
            for c in range(CT):
                nc.tensor.matmul(ps_ap, wc[:, c, :],
                                 moving[c][:, sl * NSL:(sl + 1) * NSL],
                                 start=(c == 0), stop=(c == CT - 1))

        # prephase: K(0), Q(0), V (own pools, closed before attention)
        with tc.tile_pool(name="wvp", bufs=1) as wvp, \
             tc.tile_pool(name="psA", bufs=6, space="PSUM") as psA:
            wv_sb = [wvp.tile([P, D_MODEL], bf16, name=f"wv{c}") for c in range(CT)]
            for c in range(CT):
                nc.scalar.dma_start(out=wv_sb[c], in_=wv[c * P:(c + 1) * P, :])
            wc = wc0
            kt_cur = ktq.tile([P, SEQ], bf16, tag="kt", name="kt0")
            qt_cur = ktq.tile([P, QLEN], bf16, tag="qt", name="qt0")
            for j in range(JS):
                ps = psA.tile([P, NSL], f32, tag="psa", name=f"psk0_{j}")
                kq_group(ps, wc, ht_sb, j)
                nc.vector.tensor_copy(kt_cur[:, j * NSL:(j + 1) * NSL], ps)
            wc = load_wcol(wq, 0, "wqc")
            for i in range(IS):
                ps = psA.tile([P, NSL], f32, tag="psa", name=f"psq0_{i}")
                kq_group(ps, wc, ht_sb, i)
                nc.vector.tensor_copy(qt_cur[:, i * NSL:(i + 1) * NSL], ps)
            # V projection: stationary h^T tiles, moving Wv slabs.
            # The psum->sbuf copy scales V rows by the per-key mask z (so
            # masked keys contribute nothing to PV), and the z-column (col 64
            # per head) makes the same PV matmul accumulate the softmax
            # denominator in its psum row 64.
            for t in range(JT):
                for es in range(2):
                    ps = psA.tile([P, NSL], f32, tag="psa", name=f"psv{t}_{es}")
                    for c in range(CT):
                        nc.tensor.matmul(ps, ht_sb[c][:, t * P:(t + 1) * P],
                                         wv_sb[c][:, es * NSL:(es + 1) * NSL],
                                         start=(c == 0), stop=(c == CT - 1))
                    nc.vector.tensor_scalar(
                        out=v_sb[t][:, es * 8:(es + 1) * 8, 0:D_HEAD],
                        in0=ps[:, :].rearrange("p (h d) -> p h d", d=D_HEAD),
                        scalar1=z_sb[:, t:t + 1], scalar2=None,
                        op0=ALU.mult)
                nc.vector.tensor_scalar(
                    out=v_sb[t][:, :, D_HEAD:DH1], in0=ones16,
                    scalar1=z_sb[:, t:t + 1], scalar2=None, op0=ALU.mult)

        def emit_pv(nc, v_sb, av, hp, j, pts):
            first, last = (j == 0), (j == JT - 1)
            for hb in range(2):
                h = hp * 2 + hb
                for i in range(IS):
                    sl = slice(i * NSL, (i + 1) * NSL)
                    nc.tensor.matmul(av[hb][i][0:DH1, :], v_sb[j][:, h, :],
                                     pts[hb][:, sl], start=first, stop=last,
                                     tile_position=(0, 0),
                                     skip_group_check=(hb + i > 0))

        for c in range(CT):
            nc.scalar.dma_start(out=wo_sb[c], in_=wo[c * P:(c + 1) * P, :])
        nc.gpsimd.dma_start(out=gam_sb,
                            in_=bass.AP(tensor=gam, offset=0, ap=[[0, P], [1, D_MODEL]]))
        nc.gpsimd.dma_start(out=bet_sb,
                            in_=bass.AP(tensor=bet, offset=0, ap=[[0, P], [1, D_MODEL]]))

        # ---- attention ------------------------------------------------------
        attn_ctx = ExitStack()
        scp = attn_ctx.enter_context(tc.tile_pool(name="scp", bufs=2, space="PSUM"))
        avp = attn_ctx.enter_context(tc.tile_pool(name="avp", bufs=4, space="PSUM"))
        ptp = attn_ctx.enter_context(tc.tile_pool(name="ptp", bufs=6))
        # two pairs of avc/rep are in flight at once (stage 2 of pair hp
        # runs at pair hp+1's end) — slot counts must cover both, or the
        # stage-1 writes would wait on stage-2 reads that are issued later
        # in engine program order (cross-engine deadlock).
        nrmA = attn_ctx.enter_context(tc.tile_pool(name="nrmA", bufs=8))
        nrmB = attn_ctx.enter_context(tc.tile_pool(name="nrmB", bufs=6))
        nrmC = attn_ctx.enter_context(tc.tile_pool(name="nrmC", bufs=3))
        nrmT = attn_ctx.enter_context(tc.tile_pool(name="nrmT", bufs=2))

        # Normalize runs as a two-stage software pipeline. Stage 1 (at the
        # owning pair's end): gpsimd evacuates av psum -> avc sbuf (freeing
        # the psum slots the next pair's PV needs within ~1us), then the
        # slow DVE reciprocal (~2.7us each, 5-pass microcode) of the den
        # rows is queued lazily. Stage 2 (deferred to the NEXT pair's end,
        # when the reciprocals have long finished): gpsimd broadcast of
        # 1/den to 64 rows, DVE multiply into avt (head 1 via a
        # partition-shift DMA). Nothing in the chain gates the PE.
        def norm_stage1(hp, av):
            # i-slab-major order so the i=0 column halves of avt complete
            # first (the output projection can start on them at the end).
            avcs, reps = [], []
            for i in range(IS):
                for hb in range(2):
                    avc = nrmA.tile([P, NSL], f32, tag="avc",
                                   name=f"avc{hp}_{hb}_{i}")
                    # DVE (gpsimd can't read PSUM); first in the block so
                    # the av psum slots free fast.
                    nc.vector.tensor_copy(avc[0:DH1, :], av[hb][i][0:DH1, :])
                    avcs.append(avc)
            for idx, avc in enumerate(avcs):
                rep = nrmB.tile([P, NSL], f32, tag="rep", name=f"rep{hp}_{idx}")
                # A [1,512] reciprocal runs on ONE DVE lane (~3.3us: 5-pass
                # microcode). Spread the 512 dens over 32 lanes instead:
                # 32x32-block stream transpose puts den[32b+r] at
                # tmp[r, 32b], the strided reciprocal then costs free=16,
                # and the transpose back lands 1/den on row 0. ~1.4us.
                tmp = nrmT.tile([P, NSL], f32, tag="tmp", name=f"tm{hp}_{idx}")
                nc.vector.transpose(tmp[0:32, :], avc[64:96, :])
                nc.vector.reciprocal(
                    tmp[0:32, :].rearrange("p (b c) -> p c b", c=32)[:, 0:1, :],
                    tmp[0:32, :].rearrange("p (b c) -> p c b", c=32)[:, 0:1, :])
                nc.vector.transpose(rep[0:32, :], tmp[0:32, :])
                reps.append(rep)
            return avcs, reps

        def norm_stage2(hp, avcs, reps):
            for i in range(IS):
                for hb in range(2):
                    idx = i * 2 + hb
                    sl = slice(i * NSL, (i + 1) * NSL)
                    repl = nrmC.tile([P, NSL], f32, tag="repl",
                                    name=f"repl{hp}_{hb}_{i}")
                    nc.gpsimd.partition_broadcast(repl[0:64, :],
                                                  reps[idx][0:1, :])
                    if hb == 0:
                        nc.vector.tensor_mul(avt_sb[hp][0:64, sl],
                                             avcs[idx][0:64, :],
                                             repl[0:64, :])
                    else:
                        navt = nrmC.tile([P, NSL], bf16, tag="navt",
                                        name=f"navt{hp}_{i}")
                        nc.vector.tensor_mul(navt[0:64, :],
                                             avcs[idx][0:64, :],
                                             repl[0:64, :])
                        eng = nc.sync if i == 0 else nc.scalar
                        eng.dma_start(out=avt_sb[hp][64:P, sl],
                                      in_=navt[0:64, :])

        pending = None

        for hp in range(HP):
            av = [[avp.tile([P, NSL], f32, tag="av", name=f"av{hp}_{hb}_{i}")
                   for i in range(IS)] for hb in range(2)]
            # interleaved projection work for the NEXT head pair, borrowing
            # scores-pool psum slots: (emit_at_j, which, slab)
            proj_work = {4: ("k", 0), 8: ("k", 2), 12: ("q", 0)} if hp + 1 < HP else {}
            # PV runs two j behind the scores/exp front: pt(j-2) is long done
            # when PV(j-2) issues, so the PE never waits on the ACT stream,
            # and at hp start the two-j slack absorbs the previous pair's
            # normalize latency (which frees the av psum slots).
            pt_q = []
            wc_k = None
            kt_nxt = qt_nxt = None

            for j in range(JT):
                cur_pt = []
                for hb in range(2):
                    base = hb * 64
                    sc = scp.tile([P, QLEN], f32, tag="sc", name=f"sc{hp}_{j}_{hb}")
                    for i in range(IS):
                        nc.tensor.matmul(
                            sc[:, i * NSL:(i + 1) * NSL],
                            kt_cur[base:base + 64, j * P:(j + 1) * P],
                            qt_cur[base:base + 64, i * NSL:(i + 1) * NSL],
                            start=True, stop=True, tile_position=(base, 0))
                    pt_t = ptp.tile([P, QLEN], bf16, tag="pt",
                                    name=f"pt{hp}_{j}_{hb}")
                    nc.scalar.activation(pt_t, sc, AF.Exp, scale=SCALE)
                    cur_pt.append(pt_t)

                pt_q.append(cur_pt)
                if j >= 2:
                    emit_pv(nc, v_sb, av, hp, j - 2, pt_q[j - 2])

                if j in proj_work:
                    kind, sl0 = proj_work[j]
                    borrow = scp.tile([P, QLEN], f32, tag="sc",
                                      name=f"bw{hp}_{j}")
                    if kind == "k":
                        if sl0 == 0:
                            wc_k = load_wcol(wk, hp + 1, "wkc")
                            kt_nxt = ktq.tile([P, SEQ], bf16, tag="kt",
                                              name=f"kt{hp + 1}")
                        for g in range(2):
                            sl = sl0 + g
                            kq_group(borrow[:, g * NSL:(g + 1) * NSL],
                                     wc_k, ht_sb, sl)
                            nc.vector.tensor_copy(
                                kt_nxt[:, sl * NSL:(sl + 1) * NSL],
                                borrow[:, g * NSL:(g + 1) * NSL])
                    else:
                        wc_q = load_wcol(wq, hp + 1, "wqc")
                        qt_nxt = ktq.tile([P, QLEN], bf16, tag="qt",
                                          name=f"qt{hp + 1}")
                        for g in range(IS):
                            kq_group(borrow[:, g * NSL:(g + 1) * NSL],
                                     wc_q, ht_sb, g)
                            nc.vector.tensor_copy(
                                qt_nxt[:, g * NSL:(g + 1) * NSL],
                                borrow[:, g * NSL:(g + 1) * NSL])

            # drain the two-deep PV pipeline
            emit_pv(nc, v_sb, av, hp, JT - 2, pt_q[JT - 2])
            emit_pv(nc, v_sb, av, hp, JT - 1, pt_q[JT - 1])

            # evacuate THIS pair's av psum first (frees the slots — the
            # output-projection psum aliases these banks, so fast
            # evacuation matters even for the last pair), then complete
            # the PREVIOUS pair's deferred normalize.
            cur = norm_stage1(hp, av)
            if pending is not None:
                norm_stage2(hp - 1, *pending)
            pending = cur
            if kt_nxt is not None:
                kt_cur, qt_cur = kt_nxt, qt_nxt

        norm_stage2(HP - 1, *pending)

        # ---- output projection + residual + layernorm -----------------------
        attn_ctx.close()
        ph1_ctx.close()

        pso = ctx.enter_context(tc.tile_pool(name="pso", bufs=8, space="PSUM"))
        lnp = ctx.enter_context(tc.tile_pool(name="lnp", bufs=3))
        lns = ctx.enter_context(tc.tile_pool(name="lns", bufs=8))

        for t in range(TQ):
            hq_t = lnp.tile([P, D_MODEL], f32, tag="hq", name=f"hq{t}")
            nc.sync.dma_start(out=hq_t, in_=hq[t * P:(t + 1) * P, :])
            xs = lnp.tile([P, D_MODEL], f32, tag="xs", name=f"xs{t}")
            sums = lns.tile([P, 2], f32, tag="sm", name=f"sm{t}")
            for m in range(2):
                ps = pso.tile([P, NSL], f32, tag="po", name=f"po{t}_{m}")
                for e in range(ET):
                    nc.tensor.matmul(ps, avt_sb[e][:, t * P:(t + 1) * P],
                                     wo_sb[e][:, m * NSL:(m + 1) * NSL],
                                     start=(e == 0), stop=(e == ET - 1))
                nc.vector.scalar_tensor_tensor(
                    out=xs[:, m * NSL:(m + 1) * NSL], in0=ps, scalar=1.0,
                    in1=hq_t[:, m * NSL:(m + 1) * NSL],
                    op0=ALU.mult, op1=ALU.add,
                    accum_out=sums[:, m:m + 1])
            # mean/var via accum sums + ACT Square pass (keeps the tail off
            # the DVE): mean = (s0+s1)/D; var = sq/D - mean^2
            sq = lns.tile([P, 2], f32, tag="sq", name=f"sq{t}")
            xsq = lnp.tile([P, D_MODEL], f32, tag="xq", name=f"xq{t}")
            for m in range(2):
                nc.scalar.activation(xsq[:, m * NSL:(m + 1) * NSL],
                                     xs[:, m * NSL:(m + 1) * NSL], AF.Square,
                                     accum_out=sq[:, m:m + 1])
            mean = lns.tile([P, 1], f32, tag="mn", name=f"mn{t}")
            nc.vector.tensor_add(mean, sums[:, 0:1], sums[:, 1:2])
            nc.vector.tensor_scalar_mul(mean, mean, 1.0 / D_MODEL)
            msq = lns.tile([P, 1], f32, tag="mq", name=f"mq{t}")
            nc.vector.tensor_mul(msq, mean, mean)
            var = lns.tile([P, 1], f32, tag="vr", name=f"vr{t}")
            nc.vector.tensor_add(var, sq[:, 0:1], sq[:, 1:2])
            nc.vector.scalar_tensor_tensor(
                out=var, in0=var, scalar=1.0 / D_MODEL, in1=msq,
                op0=ALU.mult, op1=ALU.subtract)
            std = lns.tile([P, 1], f32, tag="sd", name=f"sd{t}")
            nc.scalar.activation(std, var, AF.Sqrt, bias=eps_sb[:, 0:1])
            rstd = lns.tile([P, 1], f32, tag="rs", name=f"rs{t}")
            nc.vector.reciprocal(rstd, std)
            nmr = lns.tile([P, 1], f32, tag="nm", name=f"nm{t}")
            nc.vector.tensor_scalar_mul(nmr, mean, -1.0)
            # gs on ACT (Copy with per-partition scale), xg on gpsimd: the
            # tail is DVE-bound otherwise (~4.5us of DVE per tile).
            gs = lnp.tile([P, D_MODEL], f32, tag="gs", name=f"gs{t}")
            nc.scalar.activation(gs, gam_sb, AF.Copy, scale=rstd[:, 0:1])
            xg = lnp.tile([P, D_MODEL], f32, tag="xg", name=f"xg{t}")
            nc.vector.scalar_tensor_tensor(
                out=xg, in0=xs, scalar=nmr[:, 0:1], in1=gs,
                op0=ALU.add, op1=ALU.mult)
            xn = lnp.tile([P, D_MODEL], f32, tag="xn", name=f"xn{t}")
            nc.gpsimd.tensor_add(xn, xg, bet_sb)
            nc.sync.dma_start(out=out[t * P:(t + 1) * P, :], in_=xn)

    nc.compile()
    return nc


def _get_nc():
    if "nc" not in _CACHE:
        _CACHE["nc"] = _build()
    return _CACHE["nc"]


def _make_in_maps(inputs):
    bf = ml_dtypes.bfloat16
    h = np.asarray(inputs["h"], dtype=np.float32)
    mask = np.asarray(inputs["attn_mask"])
    Wq = np.asarray(inputs["Wq"], dtype=np.float32)
    Wkv = np.asarray(inputs["Wkv"], dtype=np.float32)
    Wo = np.asarray(inputs["Wo"], dtype=np.float32)
    gamma = np.asarray(inputs["gamma"], dtype=np.float32)
    beta = np.asarray(inputs["beta"], dtype=np.float32)

    wq_b = np.ascontiguousarray(Wq.astype(bf))
    wk_b = np.ascontiguousarray(Wkv[:, :D_MODEL].astype(bf))
    wv_b = np.ascontiguousarray(Wkv[:, D_MODEL:].astype(bf))
    wo_b = np.ascontiguousarray(Wo.astype(bf))

    in_maps = []
    for c in range(8):
        b, half = divmod(c, 2)
        hb = h[:, b, :]
        hT_b = hb.T.astype(bf)
        own = slice(half * QLEN, (half + 1) * QLEN)
        other = slice((1 - half) * QLEN, (2 - half) * QLEN)
        # own query-half first: keys are in core-local order, so the Q
        # projection can read the first half of hT uniformly on every core.
        # The mask is reordered identically; attention is key-order-invariant.
        hT_r = np.ascontiguousarray(np.concatenate(
            [hT_b[:, own], hT_b[:, other]], axis=1))
        z_full = np.where(mask[:, b], np.float32(0.0), np.float32(1.0))
        in_maps.append({
            "hT": hT_r,
            "hq": np.ascontiguousarray(hb[own, :]),
            "wq": wq_b, "wk": wk_b, "wv": wv_b, "wo": wo_b,
            "zt": np.ascontiguousarray(
                np.concatenate([z_full[own], z_full[other]])),
            "gam": gamma, "bet": beta,
        })
    return in_maps


def _run(in_maps, **kwargs):
    from concourse.bass_utils import run_bass_kernel_spmd
    return run_bass_kernel_spmd(_get_nc(), in_maps, core_ids=list(range(8)),
                                **kwargs)


def kernel(**inputs) -> np.ndarray:
    res = _run(_make_in_maps(inputs))
    out = np.empty((SEQ, BSZ, D_MODEL), dtype=np.float32)
    for c in range(8):
        b, half = divmod(c, 2)
        out[half * QLEN:(half + 1) * QLEN, :, :][:, b, :] = res.results[c]["out"]
    return out


# revision 36
# speedup vs baseline: 1.0091x; 1.0091x over previous
"""MultiHeadAttn (post-LN, key-padding mask) Trainium2 Bass kernel, 8 cores.

Problem: h [S=2048, B=4, D=1024] f32; 16 heads x 64; key-padding mask [S, B];
out = LayerNorm(h + MHA(h)).

Sharding: core c handles batch b = c//2 and query half qh = c%2 (1024 query
rows), with all 16 heads and the full 2048-key context for that batch.
KV projections are recomputed by both cores of a batch pair (no collectives).

Per-core device pipeline (all matmuls bf16, fp32 accumulation in PSUM):
  - K^T/Q^T proj: stationary Wk/Wq column tiles, moving h^T -> [e, t] layout.
  - V proj: stationary h^T tiles, moving Wv -> natural [t, e] layout. The
    key-padding mask enters here only: V rows are scaled by z in {0,1}
    (masked key -> 0) and a z-column is appended per head (col 64), so the
    PV matmul (stationary [z*V | z], M=65) accumulates both the numerator
    and, in psum row 64, the softmax denominator sum_j z_j exp(s_qj).
  - Attention per head pair: scores^T [j,i] via row-paired matmuls (two heads
    in row strips 0-63 / 64-127 of the PE array), unmasked exp via ScalarE
    with 1/sqrt(dh) as the scale, then the M=65 PV above. No separate
    denominator matmuls (the baseline spent ~109us of PE on them). PV runs
    two j behind the scores/exp front so the PE never waits on the ACT
    stream (waiting throttles the PE to its half-clock p-state: HAM k=4).
  - Normalize (two-stage software pipeline, all off the PE critical path):
    stage 1 evacuates av psum via DVE copies (the next pair's PV needs the
    banks within ~1us) and computes 1/den with a 32-lane-parallel
    transpose/strided-reciprocal/transpose (a [1,512] reciprocal is 5-pass
    microcode on ONE lane, ~3.3us; this is ~1.4us); stage 2 - deferred to
    the NEXT pair's end - broadcasts 1/den to 64 rows on the otherwise-idle
    GPSIMD (partition_broadcast) and multiplies into avt (head1 via a small
    partition-shift DMA into avt rows 64-127).
  - Output proj: stationary attn_vec^T tiles, moving Wo; residual add + LN
    spread over DVE (stt/accum), ScalarE (Square, Sqrt, gamma*rstd) and
    GPSIMD (+beta); 8 psum banks + 3-deep tile pipeline hide the chain.
Next head pair's K/Q projections are interleaved into the attention loop
(borrowing scores-pool PSUM slots) so the PE stays busy under the ACT-bound
softmax stream.
"""
import numpy as np
import ml_dtypes

N_HEAD, D_MODEL, D_HEAD = 16, 1024, 64
SEQ, BSZ = 2048, 4
QLEN = SEQ // 2
SCALE = 1.0 / D_HEAD ** 0.5
LN_EPS = 1e-5
P = 128
NSL = 512                   # matmul moving-operand slab (one PSUM bank fp32)
CT = D_MODEL // P           # 8 contraction tiles
ET = D_MODEL // P           # 8 e-tiles (2 heads each)
JT = SEQ // P               # 16 key tiles
JS = SEQ // NSL             # 4 key slabs
IS = QLEN // NSL            # 2 query slabs
TQ = QLEN // P              # 8 query-row tiles
HP = N_HEAD // 2            # 8 head pairs
DH1 = D_HEAD + 1            # V columns per head incl. the z (denominator) col

_CACHE = {}


def _build():
    from contextlib import ExitStack
    import concourse.bass as bass
    import concourse.mybir as mybir
    import concourse.tile as tile
    from concourse import bacc

    dt = mybir.dt
    f32, bf16 = dt.float32, dt.bfloat16
    AF = mybir.ActivationFunctionType
    ALU = mybir.AluOpType

    nc = bacc.Bacc(None, target_bir_lowering=False)

    hT = nc.dram_tensor("hT", [D_MODEL, SEQ], bf16, kind="ExternalInput")
    hq = nc.dram_tensor("hq", [QLEN, D_MODEL], f32, kind="ExternalInput")
    wq = nc.dram_tensor("wq", [D_MODEL, D_MODEL], bf16, kind="ExternalInput")
    wk = nc.dram_tensor("wk", [D_MODEL, D_MODEL], bf16, kind="ExternalInput")
    wv = nc.dram_tensor("wv", [D_MODEL, D_MODEL], bf16, kind="ExternalInput")
    wo = nc.dram_tensor("wo", [D_MODEL, D_MODEL], bf16, kind="ExternalInput")
    zt = nc.dram_tensor("zt", [SEQ], f32, kind="ExternalInput")
    gam = nc.dram_tensor("gam", [D_MODEL], f32, kind="ExternalInput")
    bet = nc.dram_tensor("bet", [D_MODEL], f32, kind="ExternalInput")
    out = nc.dram_tensor("out", [QLEN, D_MODEL], f32, kind="ExternalOutput")

    with tile.TileContext(nc) as tc, ExitStack() as ctx:
        persist = ctx.enter_context(tc.tile_pool(name="persist", bufs=1))

        # K/Q tiles die after their pair's scores — roll through 2 slots
        # instead of keeping all 8 pairs resident (saves 30KB of SBUF).
        ktq = ctx.enter_context(tc.tile_pool(name="ktq", bufs=2))
        v_sb = [persist.tile([P, N_HEAD, DH1], bf16, name=f"v{t}")
                for t in range(JT)]
        avt_sb = [persist.tile([P, QLEN], bf16, name=f"avt{e}") for e in range(ET)]
        z_sb = persist.tile([P, JT], f32, name="zmask")
        ones16 = persist.tile([P, N_HEAD, 1], f32, name="ones16")
        eps_sb = persist.tile([P, 1], f32, name="eps")

        nc.vector.memset(eps_sb, LN_EPS)
        nc.vector.memset(ones16, 1.0)

        nc.gpsimd.dma_start(out=z_sb,
                            in_=bass.AP(tensor=zt, offset=0, ap=[[1, P], [P, JT]]))

        # ---- phase-3 weights: load early into the region wvp freed ---------
        w3p = ctx.enter_context(tc.tile_pool(name="w3p", bufs=1))
        wo_sb = [w3p.tile([P, D_MODEL], bf16, name=f"wo{c}") for c in range(CT)]
        gam_sb = w3p.tile([P, D_MODEL], f32, name="gamr")
        bet_sb = w3p.tile([P, D_MODEL], f32, name="betr")

        # ---- phase 1 scope: h^T residency + streamed W columns --------------
        ph1_ctx = ExitStack()
        ph1 = ph1_ctx.enter_context(tc.tile_pool(name="ph1", bufs=1))
        ht_sb = [ph1.tile([P, SEQ], bf16, name=f"ht{c}") for c in range(CT)]

        wcol = ph1_ctx.enter_context(tc.tile_pool(name="wcol", bufs=3))

        def load_wcol(w, e, tag):
            wc = wcol.tile([P, CT, P], bf16, tag=tag, name=f"{tag}{e}")
            nc.sync.dma_start(
                out=wc,
                in_=w[:, e * P:(e + 1) * P].rearrange("(ct p) e -> p ct e", p=P))
            return wc

        # startup DMA priority: the first K-proj matmul needs wkc(0) + ht
        # tiles, so those go first on the sync queue; htq (needed later, for
        # Q-proj) goes via gpsimd SWDGE in parallel.
        wc0 = load_wcol(wk, 0, "wkc")
        ht_engs = [nc.sync, nc.scalar, nc.gpsimd]
        for c in range(CT):
            ht_engs[c % 3].dma_start(out=ht_sb[c], in_=hT[c * P:(c + 1) * P, :])

        def kq_group(ps_ap, wc, moving, sl):
            """8 accumulating matmuls: one K/Q-proj output group into psum."""
            for c in range(CT):
                nc.tensor.matmul(ps_ap, wc[:, c, :],
                                 moving[c][:, sl * NSL:(sl + 1) * NSL],
                                 start=(c == 0), stop=(c == CT - 1))

        # prephase: K(0), Q(0), V (own pools, closed before attention)
        with tc.tile_pool(name="wvp", bufs=1) as wvp, \
             tc.tile_pool(name="psA", bufs=6, space="PSUM") as psA:
            wv_sb = [wvp.tile([P, D_MODEL], bf16, name=f"wv{c}") for c in range(CT)]
            for c in range(CT):
                nc.scalar.dma_start(out=wv_sb[c], in_=wv[c * P:(c + 1) * P, :])
            wc = wc0
            kt_cur = ktq.tile([P, SEQ], bf16, tag="kt", name="kt0")
            qt_cur = ktq.tile([P, QLEN], bf16, tag="qt", name="qt0")
            for j in range(JS):
                ps = psA.tile([P, NSL], f32, tag="psa", name=f"psk0_{j}")
                kq_group(ps, wc, ht_sb, j)
                nc.vector.tensor_copy(kt_cur[:, j * NSL:(j + 1) * NSL], ps)
            wc = load_wcol(wq, 0, "wqc")
            for i in range(IS):
                ps = psA.tile([P, NSL], f32, tag="psa", name=f"psq0_{i}")
                kq_group(ps, wc, ht_sb, i)
                nc.vector.tensor_copy(qt_cur[:, i * NSL:(i + 1) * NSL], ps)
            # V projection: stationary h^T tiles, moving Wv slabs.
            # The psum->sbuf copy scales V rows by the per-key mask z (so
            # masked keys contribute nothing to PV), and the z-column (col 64
            # per head) makes the same PV matmul accumulate the softmax
            # denominator in its psum row 64.
            for t in range(JT):
                for es in range(2):
                    ps = psA.tile([P, NSL], f32, tag="psa", name=f"psv{t}_{es}")
                    for c in range(CT):
                        nc.tensor.matmul(ps, ht_sb[c][:, t * P:(t + 1) * P],
                                         wv_sb[c][:, es * NSL:(es + 1) * NSL],
                                         start=(c == 0), stop=(c == CT - 1))
                    nc.vector.tensor_scalar(
                        out=v_sb[t][:, es * 8:(es + 1) * 8, 0:D_HEAD],
                        in0=ps[:, :].rearrange("p (h d) -> p h d", d=D_HEAD),
                        scalar1=z_sb[:, t:t + 1], scalar2=None,
                        op0=ALU.mult)
                nc.vector.tensor_scalar(
                    out=v_sb[t][:, :, D_HEAD:DH1], in0=ones16,
                    scalar1=z_sb[:, t:t + 1], scalar2=None, op0=ALU.mult)

        def emit_pv(nc, v_sb, av, hp, j, pts):
            first, last = (j == 0), (j == JT - 1)
            for hb in range(2):
                h = hp * 2 + hb
                for i in range(IS):
                    sl = slice(i * NSL, (i + 1) * NSL)
                    nc.tensor.matmul(av[hb][i][0:DH1, :], v_sb[j][:, h, :],
                                     pts[hb][:, sl], start=first, stop=last,
                                     tile_position=(0, 0),
                                     skip_group_check=(hb + i > 0))

        for c in range(CT):
            nc.scalar.dma_start(out=wo_sb[c], in_=wo[c * P:(c + 1) * P, :])
        nc.gpsimd.dma_start(out=gam_sb,
                            in_=bass.AP(tensor=gam, offset=0, ap=[[0, P], [1, D_MODEL]]))
        nc.gpsimd.dma_start(out=bet_sb,
                            in_=bass.AP(tensor=bet, offset=0, ap=[[0, P], [1, D_MODEL]]))

        # ---- attention ------------------------------------------------------
        attn_ctx = ExitStack()
        scp = attn_ctx.enter_context(tc.tile_pool(name="scp", bufs=2, space="PSUM"))
        avp = attn_ctx.enter_context(tc.tile_pool(name="avp", bufs=4, space="PSUM"))
        ptp = attn_ctx.enter_context(tc.tile_pool(name="ptp", bufs=6))
        # two pairs of avc/rep are in flight at once (stage 2 of pair hp
        # runs at pair hp+1's end) — slot counts must cover both, or the
        # stage-1 writes would wait on stage-2 reads that are issued later
        # in engine program order (cross-engine deadlock).
        nrmA = attn_ctx.enter_context(tc.tile_pool(name="nrmA", bufs=8))
        nrmB = attn_ctx.enter_context(tc.tile_pool(name="nrmB", bufs=6))
        nrmC = attn_ctx.enter_context(tc.tile_pool(name="nrmC", bufs=3))
        nrmT = attn_ctx.enter_context(tc.tile_pool(name="nrmT", bufs=2))

        # Normalize runs as a two-stage software pipeline. Stage 1 (at the
        # owning pair's end): gpsimd evacuates av psum -> avc sbuf (freeing
        # the psum slots the next pair's PV needs within ~1us), then the
        # slow DVE reciprocal (~2.7us each, 5-pass microcode) of the den
        # rows is queued lazily. Stage 2 (deferred to the NEXT pair's end,
        # when the reciprocals have long finished): gpsimd broadcast of
        # 1/den to 64 rows, DVE multiply into avt (head 1 via a
        # partition-shift DMA). Nothing in the chain gates the PE.
        def norm_stage1(hp, av):
            # i-slab-major order so the i=0 column halves of avt complete
            # first (the output projection can start on them at the end).
            avcs, reps = [], []
            for i in range(IS):
                for hb in range(2):
                    avc = nrmA.tile([P, NSL], f32, tag="avc",
                                   name=f"avc{hp}_{hb}_{i}")
                    # DVE (gpsimd can't read PSUM); first in the block so
                    # the av psum slots free fast.
                    nc.vector.tensor_copy(avc[0:DH1, :], av[hb][i][0:DH1, :])
                    avcs.append(avc)
            for idx, avc in enumerate(avcs):
                rep = nrmB.tile([P, NSL], f32, tag="rep", name=f"rep{hp}_{idx}")
                # A [1,512] reciprocal runs on ONE DVE lane (~3.3us: 5-pass
                # microcode). Spread the 512 dens over 32 lanes instead:
                # 32x32-block stream transpose puts den[32b+r] at
                # tmp[r, 32b], the strided reciprocal then costs free=16,
                # and the transpose back lands 1/den on row 0. ~1.4us.
                tmp = nrmT.tile([P, NSL], f32, tag="tmp", name=f"tm{hp}_{idx}")
                nc.vector.transpose(tmp[0:32, :], avc[64:96, :])
                nc.vector.reciprocal(
                    tmp[0:32, :].rearrange("p (b c) -> p c b", c=32)[:, 0:1, :],
                    tmp[0:32, :].rearrange("p (b c) -> p c b", c=32)[:, 0:1, :])
                nc.vector.transpose(rep[0:32, :], tmp[0:32, :])
                reps.append(rep)
            return avcs, reps

        def norm_stage2(hp, avcs, reps):
            for i in range(IS):
                for hb in range(2):
                    idx = i * 2 + hb
                    sl = slice(i * NSL, (i + 1) * NSL)
                    repl = nrmC.tile([P, NSL], f32, tag="repl",
                                    name=f"repl{hp}_{hb}_{i}")
                    nc.gpsimd.partition_broadcast(repl[0:64, :],
                                                  reps[idx][0:1, :])
                    if hb == 0:
                        nc.vector.tensor_mul(avt_sb[hp][0:64, sl],
                                             avcs[idx][0:64, :],
                                             repl[0:64, :])
                    else:
                        navt = nrmC.tile([P, NSL], bf16, tag="navt",
                                        name=f"navt{hp}_{i}")
                        nc.vector.tensor_mul(navt[0:64, :],
                                             avcs[idx][0:64, :],
                                             repl[0:64, :])
                        eng = nc.sync if i == 0 else nc.scalar
                        eng.dma_start(out=avt_sb[hp][64:P, sl],
                                      in_=navt[0:64, :])

        pending = None

        for hp in range(HP):
            av = [[avp.tile([P, NSL], f32, tag="av", name=f"av{hp}_{hb}_{i}")
                   for i in range(IS)] for hb in range(2)]
            # interleaved projection work for the NEXT head pair, borrowing
            # scores-pool psum slots: (emit_at_j, which, slab)
            proj_work = {4: ("k", 0), 8: ("k", 2), 12: ("q", 0)} if hp + 1 < HP else {}
            # PV runs two j behind the scores/exp front: pt(j-2) is long done
            # when PV(j-2) issues, so the PE never waits on the ACT stream,
            # and at hp start the two-j slack absorbs the previous pair's
            # normalize latency (which frees the av psum slots).
            pt_q = []
            wc_k = None
            kt_nxt = qt_nxt = None

            for j in range(JT):
                cur_pt = []
                for hb in range(2):
                    base = hb * 64
                    sc = scp.tile([P, QLEN], f32, tag="sc", name=f"sc{hp}_{j}_{hb}")
                    for i in range(IS):
                        nc.tensor.matmul(
                            sc[:, i * NSL:(i + 1) * NSL],
                            kt_cur[base:base + 64, j * P:(j + 1) * P],
                            qt_cur[base:base + 64, i * NSL:(i + 1) * NSL],
                            start=True, stop=True, tile_position=(base, 0))
                    pt_t = ptp.tile([P, QLEN], bf16, tag="pt",
                                    name=f"pt{hp}_{j}_{hb}")
                    nc.scalar.activation(pt_t, sc, AF.Exp, scale=SCALE)
                    cur_pt.append(pt_t)

                pt_q.append(cur_pt)
                if j >= 2:
                    emit_pv(nc, v_sb, av, hp, j - 2, pt_q[j - 2])

                if j in proj_work:
                    kind, sl0 = proj_work[j]
                    borrow = scp.tile([P, QLEN], f32, tag="sc",
                                      name=f"bw{hp}_{j}")
                    if kind == "k":
                        if sl0 == 0:
                            wc_k = load_wcol(wk, hp + 1, "wkc")
                            kt_nxt = ktq.tile([P, SEQ], bf16, tag="kt",
                                              name=f"kt{hp + 1}")
                        for g in range(2):
                            sl = sl0 + g
                            kq_group(borrow[:, g * NSL:(g + 1) * NSL],
                                     wc_k, ht_sb, sl)
                            nc.vector.tensor_copy(
                                kt_nxt[:, sl * NSL:(sl + 1) * NSL],
                                borrow[:, g * NSL:(g + 1) * NSL])
                    else:
                        wc_q = load_wcol(wq, hp + 1, "wqc")
                        qt_nxt = ktq.tile([P, QLEN], bf16, tag="qt",
                                          name=f"qt{hp + 1}")
                        for g in range(IS):
                            kq_group(borrow[:, g * NSL:(g + 1) * NSL],
                                     wc_q, ht_sb, g)
                            nc.vector.tensor_copy(
                                qt_nxt[:, g * NSL:(g + 1) * NSL],
                                borrow[:, g * NSL:(g + 1) * NSL])

            # drain the two-deep PV pipeline
            emit_pv(nc, v_sb, av, hp, JT - 2, pt_q[JT - 2])
            emit_pv(nc, v_sb, av, hp, JT - 1, pt_q[JT - 1])

            # evacuate THIS pair's av psum first (frees the slots — the
            # output-projection psum aliases these banks, so fast
            # evacuation matters even for the last pair), then complete
            # the PREVIOUS pair's deferred normalize.
            cur = norm_stage1(hp, av)
            if pending is not None:
                norm_stage2(hp - 1, *pending)
            pending = cur
            if kt_nxt is not None:
                kt_cur, qt_cur = kt_nxt, qt_nxt

        norm_stage2(HP - 1, *pending)

        # ---- output projection + residual + layernorm -----------------------
        attn_ctx.close()
        ph1_ctx.close()

        pso = ctx.enter_context(tc.tile_pool(name="pso", bufs=8, space="PSUM"))
        lnp = ctx.enter_context(tc.tile_pool(name="lnp", bufs=3))
        lns = ctx.enter_context(tc.tile_pool(name="lns", bufs=8))

        for t in range(TQ):
            hq_t = lnp.tile([P, D_MODEL], f32, tag="hq", name=f"hq{t}")
            nc.sync.dma_start(out=hq_t, in_=hq[t * P:(t + 1) * P, :])
            xs = lnp.tile([P, D_MODEL], f32, tag="xs", name=f"xs{t}")
            sums = lns.tile([P, 2], f32, tag="sm", name=f"sm{t}")
            for m in range(2):
                ps = pso.tile([P, NSL], f32, tag="po", name=f"po{t}_{m}")
                for e in range(ET):
                    nc.tensor.matmul(ps, avt_sb[e][:, t * P:(t + 1) * P],
                                     wo_sb[e][:, m * NSL:(m + 1) * NSL],
                                     start=(e == 0), stop=(e == ET - 1))
                nc.vector.scalar_tensor_tensor(
                    out=xs[:, m * NSL:(m + 1) * NSL], in0=ps, scalar=1.0,
                    in1=hq_t[:, m * NSL:(m + 1) * NSL],
                    op0=ALU.mult, op1=ALU.add,
                    accum_out=sums[:, m:m + 1])
            # mean/var via accum sums + ACT Square pass (keeps the tail off
            # the DVE): mean = (s0+s1)/D; var = sq/D - mean^2
            sq = lns.tile([P, 2], f32, tag="sq", name=f"sq{t}")
            xsq = lnp.tile([P, D_MODEL], f32, tag="xq", name=f"xq{t}")
            for m in range(2):
                nc.scalar.activation(xsq[:, m * NSL:(m + 1) * NSL],
                                     xs[:, m * NSL:(m + 1) * NSL], AF.Square,
                                     accum_out=sq[:, m:m + 1])
            mean = lns.tile([P, 1], f32, tag="mn", name=f"mn{t}")
            nc.vector.tensor_add(mean, sums[:, 0:1], sums[:, 1:2])
            nc.vector.tensor_scalar_mul(mean, mean, 1.0 / D_MODEL)
            msq = lns.tile([P, 1], f32, tag="mq", name=f"mq{t}")
            nc.vector.tensor_mul(msq, mean, mean)
            var = lns.tile([P, 1], f32, tag="vr", name=f"vr{t}")
            nc.vector.tensor_add(var, sq[:, 0:1], sq[:, 1:2])
            nc.vector.scalar_tensor_tensor(
                out=var, in0=var, scalar=1.0 / D_MODEL, in1=msq,
                op0=ALU.mult, op1=ALU.subtract)
            std = lns.tile([P, 1], f32, tag="sd", name=f"sd{t}")
            nc.scalar.activation(std, var, AF.Sqrt, bias=eps_sb[:, 0:1])
            rstd = lns.tile([P, 1], f32, tag="rs", name=f"rs{t}")
            nc.vector.reciprocal(rstd, std)
            nmr = lns.tile([P, 1], f32, tag="nm", name=f"nm{t}")
            nc.vector.tensor_scalar_mul(nmr, mean, -1.0)
            # gs on ACT (Copy with per-partition scale), xg on gpsimd: the
            # tail is DVE-bound otherwise (~4.5us of DVE per tile).
            gs = lnp.tile([P, D_MODEL], f32, tag="gs", name=f"gs{t}")
            nc.scalar.activation(gs, gam_sb, AF.Copy, scale=rstd[:, 0:1])
            xg = lnp.tile([P, D_MODEL], f32, tag="xg", name=f"xg{t}")
            nc.vector.scalar_tensor_tensor(
                out=xg, in0=xs, scalar=nmr[:, 0:1], in1=gs,
                op0=ALU.add, op1=ALU.mult)
            xn = lnp.tile([P, D_MODEL], f32, tag="xn", name=f"xn{t}")
            nc.gpsimd.tensor_add(xn, xg, bet_sb)
            nc.sync.dma_start(out=out[t * P:(t + 1) * P, :], in_=xn)

    nc.compile()
    return nc


def _get_nc():
    if "nc" not in _CACHE:
        _CACHE["nc"] = _build()
    return _CACHE["nc"]


def _make_in_maps(inputs):
    bf = ml_dtypes.bfloat16
    h = np.asarray(inputs["h"], dtype=np.float32)
    mask = np.asarray(inputs["attn_mask"])
    Wq = np.asarray(inputs["Wq"], dtype=np.float32)
    Wkv = np.asarray(inputs["Wkv"], dtype=np.float32)
    Wo = np.asarray(inputs["Wo"], dtype=np.float32)
    gamma = np.asarray(inputs["gamma"], dtype=np.float32)
    beta = np.asarray(inputs["beta"], dtype=np.float32)

    wq_b = np.ascontiguousarray(Wq.astype(bf))
    wk_b = np.ascontiguousarray(Wkv[:, :D_MODEL].astype(bf))
    wv_b = np.ascontiguousarray(Wkv[:, D_MODEL:].astype(bf))
    wo_b = np.ascontiguousarray(Wo.astype(bf))

    in_maps = []
    for c in range(8):
        b, half = divmod(c, 2)
        hb = h[:, b, :]
        hT_b = hb.T.astype(bf)
        own = slice(half * QLEN, (half + 1) * QLEN)
        other = slice((1 - half) * QLEN, (2 - half) * QLEN)
        # own query-half first: keys are in core-local order, so the Q
        # projection can read the first half of hT uniformly on every core.
        # The mask is reordered identically; attention is key-order-invariant.
        hT_r = np.ascontiguousarray(np.concatenate(
            [hT_b[:, own], hT_b[:, other]], axis=1))
        z_full = np.where(mask[:, b], np.float32(0.0), np.float32(1.0))
        in_maps.append({
            "hT": hT_r,
            "hq": np.ascontiguousarray(hb[own, :]),
            "wq": wq_b, "wk": wk_b, "wv": wv_b, "wo": wo_b,
            "zt": np.ascontiguousarray(
                np.concatenate([z_full[own], z_full[other]])),
            "gam": gamma, "bet": beta,
        })
    return in_maps


def _run(in_maps, **kwargs):
    from concourse.bass_utils import run_bass_kernel_spmd
    return run_bass_kernel_spmd(_get_nc(), in_maps, core_ids=list(range(8)),
                                **kwargs)


def kernel(**inputs) -> np.ndarray:
    res = _run(_make_in_maps(inputs))
    out = np.empty((SEQ, BSZ, D_MODEL), dtype=np.float32)
    for c in range(8):
        b, half = divmod(c, 2)
        out[half * QLEN:(half + 1) * QLEN, :, :][:, b, :] = res.results[c]["out"]
    return out
